# revision 1
# baseline (speedup 1.0000x reference)
"""Trainium2 Bass kernel for nn_CrossAttentionBlock (B=2, S=2048, D=1024, H=16, HD=64).

Sharding: 8 cores = 2 batches x 4 head-quads (4 heads each, E=256 channels).
Each core computes q/k/v projections for its quad, RoPE, SDPA, and a partial
output projection [S, D]; host sums the 4 partials per batch and adds bo.

Device pipeline (all matmul operands bf16, fp32 PSUM accumulation):
  - host ships x^T and W^T (d-major) with a ones-row appended to x^T and the
    bias as an extra weight row, so biases are exact.
  - RoPE is GPT-NeoX-interleaved; we fold the even/odd channel permutation into
    the q/k weight rows on the host (dot products are permutation invariant),
    which turns it into rot-half RoPE: out = q*cos + swap_halves(q)*sin with
    sign folded into the sin table. swap_halves is a partition swap done by
    SBUF->SBUF DMA; cos/sin tables are elementwise DVE multiplies.
  - scores^T [k_, q] per head via row-tiled PE pairs (k=64 each, concurrent),
    exp on ACT (PSUM->SBUF bf16, FD=1024 per head-pair), ctx^T accumulation via
    col-tiled PE pairs, softmax denominators via ones-matmul m=1 column groups.
  - normalization of ctx by 1/l (reciprocal_approx_fast + DMA partition
    broadcast) happens before the output projection so the per-head scale is
    applied before heads are mixed.
"""
import os
import sys

sys.path.insert(0, "/opt/trn_rl_repo")

import numpy as np
import ml_dtypes

BF16 = ml_dtypes.bfloat16

B, S, D, H = 2, 2048, 1024, 16
HD = D // H          # 64
DIM = HD // 2        # 32
QUADS = 4            # head groups of 4
E = D // QUADS       # 256 channels per core
ROPE_BASE = 10000.0
N_CORES = 8

KTILES = D // 128    # 8  (plus one bias row)
ST = S // 128        # 16 s-tiles
QC = S // 512        # 4 q-chunks


def _host_prep(x_q, x_kv, wq, bq, wk, bk, wv, bv, wo):
    """Build the per-core input maps (all bf16 except noted)."""
    perm = np.concatenate([np.arange(0, HD, 2), np.arange(1, HD, 2)])  # even|odd
    scale = 1.0 / np.sqrt(HD)

    freqs = np.exp(-np.arange(DIM, dtype=np.float64) * np.log(ROPE_BASE) / DIM)
    ang = np.arange(S, dtype=np.float64)[:, None] * freqs[None, :]     # [S, 32]
    cos = np.cos(ang).T                                                # [32, S]
    sin = np.sin(ang).T
    # [e(64), s] tables for one head-block, repeated across the two heads of a
    # 128-partition pair tile.  rot-half: out = q*cos + swap(q)*sin_signed
    cos64 = np.concatenate([cos, cos], axis=0)                         # [64, S]
    sin64 = np.concatenate([-sin, sin], axis=0)
    cosT = np.concatenate([cos64, cos64], axis=0).astype(BF16)         # [128, S]
    sinT = np.concatenate([sin64, sin64], axis=0).astype(BF16)

    def proj_mat(w, b, permute, s):
        # rows for one quad stacked [256, 1024] (+bias row) -> [1025, 256] d-major
        blocks, brows = [], []
        for h in range(4):
            rows = slice(h * HD, (h + 1) * HD)
            wb = w[rows, :]
            bb = b[rows]
            if permute:
                wb = wb[perm, :]
                bb = bb[perm]
            blocks.append(wb * s)
            brows.append(bb * s)
        wstack = np.concatenate(blocks, axis=0)          # [256, 1024]
        bstack = np.concatenate(brows, axis=0)           # [256]
        return np.concatenate([wstack.T, bstack[None, :]], axis=0)  # [1025, 256]

    ones_row = np.ones((1, S), dtype=np.float32)
    in_maps = []
    for c in range(N_CORES):
        b_ = c // QUADS
        g = c % QUADS
        hs = slice(g * E, (g + 1) * E)  # channel rows of this quad
        xqT = np.concatenate([x_q[b_].T, ones_row], axis=0).astype(BF16)    # [1025,S]
        xkvT = np.concatenate([x_kv[b_].T, ones_row], axis=0).astype(BF16)
        wq_g = np.ascontiguousarray(
            proj_mat(wq[hs, :], bq[hs], True, scale)).astype(BF16)
        wk_g = np.ascontiguousarray(
            proj_mat(wk[hs, :], bk[hs], True, 1.0)).astype(BF16)
        wv_g = np.ascontiguousarray(
            proj_mat(wv[hs, :], bv[hs], False, 1.0)).astype(BF16)
        woT_g = np.ascontiguousarray(wo[:, hs].T).astype(BF16)             # [256,1024]
        in_maps.append({
            "xqT": xqT, "xkvT": xkvT,
            "wqT": wq_g, "wkT": wk_g, "wvT": wv_g, "woT": woT_g,
            "cosT": np.ascontiguousarray(cosT),
            "sinT": np.ascontiguousarray(sinT),
            "ones_col": np.ones((128, 1), dtype=BF16),
        })
    return in_maps


# ---------------------------------------------------------------------------
_PROGRAM_CACHE = {}


def _fixed_tile_context(tile_mod, bass_rust_mod, vector_clock_mod):
    """TileContext whose tail drain splits multi-sem waits into single-wait
    NOPs (this walrus rejects >1 sync-wait on one instruction)."""
    SyncInfo = bass_rust_mod.SyncInfo
    ScopedClock = vector_clock_mod.ScopedClock

    class TC(tile_mod.TileContext):
        def _drain_and_barrier(self, tick_clock, wait_clock):
            harvest = self.nc.sync.nop(nofuse=True)
            wait_clock.add_sem_waits(
                harvest.ins, ScopedClock({None: tick_clock.global_clock}))
            si = harvest.ins.sync_info
            waits = list(si.on_wait) if si is not None else []
            if len(waits) > 1:
                harvest.ins.sync_info = SyncInfo(
                    on_wait=[waits[0]], on_update=list(si.on_update))
                for w in waits[1:]:
                    nop = self.nc.sync.nop(nofuse=True)
                    nop.ins.sync_info = SyncInfo(on_wait=[w], on_update=[])
            self.nc.sync.drain()
            self.nc.all_engine_barrier()
            assert self.sems is not None
            popped = self.nc._tile_sem_poison_stack.pop()
            assert popped is self._sem_poison
            self.nc.clear_and_free_semaphores(list(self.sems.allocated().values()))
            self.nc.all_engine_barrier()

    return TC


def _split_multiwait_instructions(nc, mybir, SyncInfo):
    """This walrus build rejects >1 sync-wait per instruction; hoist extra
    waits onto single-wait NOPs inserted just before, on the same engine."""
    ctr = 0
    for blk in nc.m.functions[0].blocks:
        insts = blk.instructions
        i = 0
        while i < len(insts):
            inst = insts[i]
            si = inst.sync_info
            if si is not None and len(si.on_wait) > 1:
                waits = list(si.on_wait)
                inst.sync_info = SyncInfo(on_wait=[waits[-1]],
                                          on_update=list(si.on_update))
                nops = []
                for w in waits[:-1]:
                    nop = mybir.InstNoOp(name=f"waitsplit_{ctr}", ins=[], outs=[])
                    ctr += 1
                    nop.engine = inst.engine
                    nop.sync_info = SyncInfo(on_wait=[w], on_update=[])
                    nops.append(nop)
                insts[i:i] = nops
                i += len(nops)
            i += 1
    return ctr


def build_program(split_waits=True):
    import concourse.bass as bass
    import concourse.mybir as mybir
    import concourse.tile as tile
    import bass_rust
    from concourse import vector_clock
    from concourse import library_config

    f32 = mybir.dt.float32
    bf16 = mybir.dt.bfloat16
    Exp = mybir.ActivationFunctionType.Exp
    Copy = mybir.ActivationFunctionType.Copy
    mult = mybir.AluOpType.mult
    add = mybir.AluOpType.add

    nc = bass.Bass("TRN2", target_bir_lowering=False, debug=False,
                   num_devices=N_CORES)

    xqT = nc.dram_tensor("xqT", [D + 1, S], bf16, kind="ExternalInput").ap()
    xkvT = nc.dram_tensor("xkvT", [D + 1, S], bf16, kind="ExternalInput").ap()
    wqT = nc.dram_tensor("wqT", [D + 1, E], bf16, kind="ExternalInput").ap()
    wkT = nc.dram_tensor("wkT", [D + 1, E], bf16, kind="ExternalInput").ap()
    wvT = nc.dram_tensor("wvT", [D + 1, E], bf16, kind="ExternalInput").ap()
    woT = nc.dram_tensor("woT", [E, D], bf16, kind="ExternalInput").ap()
    cosT = nc.dram_tensor("cosT", [128, S], bf16, kind="ExternalInput").ap()
    sinT = nc.dram_tensor("sinT", [128, S], bf16, kind="ExternalInput").ap()
    ones_col = nc.dram_tensor("ones_col", [128, 1], bf16, kind="ExternalInput").ap()
    out = nc.dram_tensor("out", [S, D], f32, kind="ExternalOutput").ap()

    TC = _fixed_tile_context(tile, bass_rust, vector_clock)

    with TC(nc) as tc:
        with tc.tile_pool(name="persist", bufs=1) as per:
            # ---- load inputs ----
            xq_sb = per.tile([128, KTILES * S], bf16, tag="xq")
            xkv_sb = per.tile([128, KTILES * S], bf16, tag="xkv")
            xqb_sb = per.tile([1, S], bf16, tag="xqb")       # ones rows
            xkvb_sb = per.tile([1, S], bf16, tag="xkvb")
            for k in range(KTILES):
                nc.sync.dma_start(xq_sb[:, k * S:(k + 1) * S],
                                  xqT[k * 128:(k + 1) * 128, :])
                nc.sync.dma_start(xkv_sb[:, k * S:(k + 1) * S],
                                  xkvT[k * 128:(k + 1) * 128, :])
            nc.sync.dma_start(xqb_sb[:, :], xqT[D:D + 1, :])
            nc.sync.dma_start(xkvb_sb[:, :], xkvT[D:D + 1, :])

            wq_sb = per.tile([128, KTILES * E], bf16, tag="wq")
            wk_sb = per.tile([128, KTILES * E], bf16, tag="wk")
            wv_sb = per.tile([128, KTILES * E], bf16, tag="wv")
            wqb_sb = per.tile([1, E], bf16, tag="wqb")
            wkb_sb = per.tile([1, E], bf16, tag="wkb")
            wvb_sb = per.tile([1, E], bf16, tag="wvb")
            for w_sb, wb_sb, w_dram in ((wq_sb, wqb_sb, wqT),
                                        (wk_sb, wkb_sb, wkT),
                                        (wv_sb, wvb_sb, wvT)):
                for k in range(KTILES):
                    nc.sync.dma_start(w_sb[:, k * E:(k + 1) * E],
                                      w_dram[k * 128:(k + 1) * 128, :])
                nc.sync.dma_start(wb_sb[:, :], w_dram[D:D + 1, :])

            wo_sb = per.tile([128, 2 * D], bf16, tag="wo")   # pair p at cols p*D
            for p in range(2):
                nc.sync.dma_start(wo_sb[:, p * D:(p + 1) * D],
                                  woT[p * 128:(p + 1) * 128, :])
            cos_sb = per.tile([128, S], bf16, tag="cos")
            sin_sb = per.tile([128, S], bf16, tag="sin")
            nc.sync.dma_start(cos_sb[:, :], cosT[:, :])
            nc.sync.dma_start(sin_sb[:, :], sinT[:, :])
            ones_sb = per.tile([128, 1], bf16, tag="ones")
            nc.sync.dma_start(ones_sb[:, :], ones_col[:, :])

            # persistent activations
            qr_sb = [per.tile([128, S], bf16, tag=f"qr{p}", name=f"qr{p}") for p in range(2)]
            kr_sb = [per.tile([128, S], bf16, tag=f"kr{p}", name=f"kr{p}") for p in range(2)]
            v_sb = per.tile([128, ST * E], bf16, tag="v")    # s-tile st at cols st*E
            ctxn_sb = [per.tile([128, S], bf16, tag=f"ctxn{p}", name=f"ctxn{p}") for p in range(2)]

            # ---- phase A: projections + rope ----
            def qk_projection(w_sb_, wb_sb_, dst, is_q):
                # dst[p][e(128), s] for pair p; rope applied
                with tc.tile_pool(name="qk_ps", bufs=2, space="PSUM") as pps, \
                     tc.tile_pool(name="qk_tmp", bufs=2) as tmp:
                    for p in range(2):
                        q_ps = pps.tile([128, S], f32, tag="q_ps")
                        for sc in range(QC):
                            ss = slice(sc * 512, (sc + 1) * 512)
                            for k in range(KTILES):
                                nc.tensor.matmul(
                                    q_ps[:, ss],
                                    lhsT=w_sb_[:, k * E + p * 128: k * E + (p + 1) * 128],
                                    rhs=(xq_sb if is_q else xkv_sb)[:, k * S + sc * 512:
                                                                    k * S + (sc + 1) * 512],
                                    start=(k == 0), stop=False)
                            nc.tensor.matmul(
                                q_ps[:, ss],
                                lhsT=wb_sb_[:, p * 128:(p + 1) * 128],
                                rhs=(xqb_sb if is_q else xkvb_sb)[:, ss],
                                start=False, stop=True)
                        qb = tmp.tile([128, S], bf16, tag="qb")
                        qsw = tmp.tile([128, S], bf16, tag="qsw")
                        qcos = tmp.tile([128, S], bf16, tag="qcos")
                        nc.scalar.activation(qb[:, :], q_ps[:, :], Copy)
                        # swap halves within each 64-block (partition swap, DMA)
                        for a, bdst in ((0, 32), (32, 0), (64, 96), (96, 64)):
                            nc.sync.dma_start(qsw[bdst:bdst + 32, :], qb[a:a + 32, :])
                        nc.vector.tensor_tensor(qcos[:, :], qb[:, :], cos_sb[:, :], mult)
                        nc.vector.tensor_tensor(qsw[:, :], qsw[:, :], sin_sb[:, :], mult)
                        nc.vector.tensor_tensor(dst[p][:, :], qcos[:, :], qsw[:, :], add)

            qk_projection(wq_sb, wqb_sb, qr_sb, True)
            qk_projection(wk_sb, wkb_sb, kr_sb, False)

            with tc.tile_pool(name="v_ps", bufs=2, space="PSUM") as vps:
                for st in range(ST):
                    v_ps = vps.tile([128, E], f32, tag="v_ps")
                    for k in range(KTILES):
                        nc.tensor.matmul(
                            v_ps[:, :],
                            lhsT=xkv_sb[:, k * S + st * 128: k * S + (st + 1) * 128],
                            rhs=wv_sb[:, k * E:(k + 1) * E],
                            start=(k == 0), stop=False)
                    nc.tensor.matmul(
                        v_ps[:, :],
                        lhsT=xkvb_sb[:, st * 128:(st + 1) * 128],
                        rhs=wvb_sb[:, :],
                        start=False, stop=True)
                    nc.scalar.activation(v_sb[:, st * E:(st + 1) * E], v_ps[:, :], Copy)

            # ---- phase B: SDPA ----
            with tc.tile_pool(name="sc_ps", bufs=2, space="PSUM") as scp, \
                 tc.tile_pool(name="cd_ps", bufs=1, space="PSUM") as cdp, \
                 tc.tile_pool(name="e_sb", bufs=3) as esp, \
                 tc.tile_pool(name="norm", bufs=2) as nrm, \
                 tc.tile_pool(name="ldram", bufs=2, space="DRAM") as ldr:
                for qh in range(QC):
                    qs = slice(qh * 512, (qh + 1) * 512)
                    ctx_ps = [cdp.tile([128, 512], f32, tag=f"ctx{p}", name=f"ctx{p}") for p in range(2)]
                    den_ps = cdp.tile([128, 512], f32, tag="den")
                    nc.vector.memset(den_ps[:, :], 1.0)
                    e_tiles = [None, None]
                    for ki in range(ST):
                        ks = slice(ki * 128, (ki + 1) * 128)
                        for p in range(2):
                            s_ps = scp.tile([128, 1024], f32, tag="s")
                            nc.tensor.matmul(
                                s_ps[:, 0:512],
                                lhsT=kr_sb[p][0:64, ks], rhs=qr_sb[p][0:64, qs],
                                tile_position=(0, 0), start=True, stop=True)
                            nc.tensor.matmul(
                                s_ps[:, 512:1024],
                                lhsT=kr_sb[p][64:128, ks], rhs=qr_sb[p][64:128, qs],
                                tile_position=(64, 0), start=True, stop=True)
                            e_sb = esp.tile([128, 1024], bf16, tag=f"e{p}")
                            nc.scalar.activation(e_sb[:, :], s_ps[:, :], Exp)
                            e_tiles[p] = e_sb
                            nc.tensor.matmul(
                                ctx_ps[p][0:64, :],
                                lhsT=v_sb[:, ki * E + (2 * p) * 64: ki * E + (2 * p) * 64 + 64],
                                rhs=e_sb[:, 0:512],
                                tile_position=(0, 0),
                                start=(ki == 0), stop=(ki == ST - 1),
                                skip_group_check=True)
                            nc.tensor.matmul(
                                ctx_ps[p][64:128, :],
                                lhsT=v_sb[:, ki * E + (2 * p + 1) * 64: ki * E + (2 * p + 1) * 64 + 64],
                                rhs=e_sb[:, 512:1024],
                                tile_position=(0, 64),
                                start=(ki == 0), stop=(ki == ST - 1),
                                skip_group_check=True)
                        # denominators: 4 heads, one col group each
                        for g, (p, half) in enumerate(((0, 0), (0, 1), (1, 0), (1, 1))):
                            nc.tensor.matmul(
                                den_ps[g * 32: g * 32 + 1, :],
                                lhsT=ones_sb[:, :],
                                rhs=e_tiles[p][:, half * 512:(half + 1) * 512],
                                tile_position=(0, g * 32),
                                start=(ki == 0), stop=(ki == ST - 1),
                                skip_group_check=True)
                    # normalize: linv rows -> DRAM roundtrip broadcast -> ctx * linv
                    linv = nrm.tile([128, 512], f32, tag="linv")
                    nc.vector.reciprocal(linv[:, :], den_ps[:, :])
                    lscr = ldr.tile([4, 512], f32, tag="lscr")
                    nc.sync.dma_start(
                        lscr[:, :], linv[0:128:32, :])
                    lbc = [nrm.tile([128, 512], f32, tag=f"lbc{p}", name=f"lbc{p}") for p in range(2)]
                    for g, (p, half) in enumerate(((0, 0), (0, 1), (1, 0), (1, 1))):
                        nc.sync.dma_start(
                            lbc[p][half * 64:(half + 1) * 64, :],
                            lscr[g:g + 1, :].partition_broadcast(64))
                    for p in range(2):
                        nc.vector.tensor_tensor(
                            ctxn_sb[p][:, qs], ctx_ps[p][:, :], lbc[p][:, :], mult)

            # ---- phase C: output projection ----
            with tc.tile_pool(name="o_ps", bufs=2, space="PSUM") as ops, \
                 tc.tile_pool(name="o_sb", bufs=2) as osb:
                for st in range(ST):
                    o_ps = ops.tile([128, D], f32, tag="o")
                    for ch in range(2):
                        cs = slice(ch * 512, (ch + 1) * 512)
                        for p in range(2):
                            nc.tensor.matmul(
                                o_ps[:, cs],
                                lhsT=ctxn_sb[p][:, st * 128:(st + 1) * 128],
                                rhs=wo_sb[:, p * D + ch * 512: p * D + (ch + 1) * 512],
                                start=(p == 0), stop=(p == 1))
                    o_out = osb.tile([128, D], f32, tag="oo")
                    nc.vector.tensor_copy(o_out[:, :], o_ps[:, :])
                    nc.sync.dma_start(out[st * 128:(st + 1) * 128, :], o_out[:, :])

    if split_waits:
        _split_multiwait_instructions(nc, mybir, bass_rust.SyncInfo)
    return nc


def kernel(x_q, x_kv, wq, bq, wk, bk, wv, bv, wo, bo):
    from concourse import bass_utils

    x_q = np.asarray(x_q, dtype=np.float32)
    x_kv = np.asarray(x_kv, dtype=np.float32)
    wq = np.asarray(wq, dtype=np.float32); bq = np.asarray(bq, dtype=np.float32)
    wk = np.asarray(wk, dtype=np.float32); bk = np.asarray(bk, dtype=np.float32)
    wv = np.asarray(wv, dtype=np.float32); bv = np.asarray(bv, dtype=np.float32)
    wo = np.asarray(wo, dtype=np.float32); bo = np.asarray(bo, dtype=np.float32)

    in_maps = _host_prep(x_q, x_kv, wq, bq, wk, bk, wv, bv, wo)

    if "prog" not in _PROGRAM_CACHE:
        _PROGRAM_CACHE["prog"] = build_program()
    nc = _PROGRAM_CACHE["prog"]

    res = bass_utils.run_bass_kernel_spmd(
        nc, in_maps, core_ids=list(range(N_CORES)),
        trace=os.environ.get("KERNEL_TRACE", "") == "1")
    _PROGRAM_CACHE["last_result"] = res

    out = np.zeros((B, S, D), dtype=np.float32)
    for c in range(N_CORES):
        out[c // QUADS] += res.results[c]["out"]
    out += bo[None, None, :]
    return out



# revision 6
# speedup vs baseline: 1.0234x; 1.0234x over previous
"""Trainium2 Bass kernel for nn_CrossAttentionBlock (B=2, S=2048, D=1024, H=16, HD=64).

Sharding: 8 cores = 2 batches x 4 head-quads (4 heads each, E=256 channels).
Each core computes q/k/v projections for its quad, RoPE, SDPA, and a partial
output projection [S, D]; host sums the 4 partials per batch and adds bo
(and, when biases are nonzero, host-correctable bias terms).

Pipeline (all matmul operands bf16, fp32 PSUM accumulation):
  - input DMAs are split per k-tile across two HWDGE queues (sync: wk+xkv,
    scalar: wv/wq/cos/sin/xq/wo) so the K projection starts ~4us in and
    chases the DMA stream (k-OUTER accumulation, both pairs resident in PSUM).
  - RoPE is GPT-NeoX-interleaved; the even/odd channel permutation is folded
    into the q/k weight rows on the host, turning it into rot-half RoPE:
    out = q*cos + swap_halves(q)*sin with sign folded into the sin table.
  - SDPA per (q-chunk, k-tile): scores^T [k,q] per head-pair via row-tiled
    concurrent PE pairs (contraction 64 each); exp is SPLIT across engines:
    pair 0 -> ACT (true exp, PSUM->SBUF bf16), pair 1 -> DVE via the
    Schraudolph bit-trick (int16(A*x + B) bitcast to bf16, ~1% rms rel err
    that diffuses to ~0 after softmax-weighted averaging); ctx^T accumulated
    via col-tiled concurrent PE pairs; softmax denominators via a col-tiled
    ones-matmul pass.
  - per q-chunk: ctx is copied out of PSUM unnormalized (ACT, bf16) so the
    PSUM banks recycle immediately; 1/den via reciprocal_approx_fast, a
    strided-DMA extract (SWDGE, fp32->bf16 cast) + partition-broadcast
    restores per-head rows; normalization is a bf16 DVE multiply.
  - the output projection for q-chunk i is interleaved into the SDPA of
    chunk i+1 (PSUM slot sharing with the scores pool), and the [S,D] fp32
    result streams back per 128-row tile.
  - V projection (resident xkv) is interleaved with the first q-chunk's SDPA.
"""
import os
import sys

sys.path.insert(0, "/opt/trn_rl_repo")

import numpy as np
import ml_dtypes

BF16 = ml_dtypes.bfloat16

B, S, D, H = 2, 2048, 1024, 16
HD = D // H          # 64
DIM = HD // 2        # 32
QUADS = 4            # head groups of 4
E = D // QUADS       # 256 channels per core
ROPE_BASE = 10000.0
N_CORES = 8

KTILES = D // 128    # 8
ST = S // 128        # 16 s-tiles
QC = S // 512        # 4 q-chunks
KI = S // 128        # 16 k-tiles per SDPA chunk

# Schraudolph bf16 exp: bits = int16(A*x + (B - sigma)); bitcast to bf16.
EXP_A = 128.0 / float(np.log(2.0))
EXP_B = 127.0 * 128.0
EXP_SIGMA = 2.0


def _host_prep(x_q, x_kv, wq, bq, wk, bk, wv, bv, wo):
    """Build the per-core input maps (all bf16)."""
    perm = np.concatenate([np.arange(0, HD, 2), np.arange(1, HD, 2)])  # even|odd
    scale = 1.0 / np.sqrt(HD)

    freqs = np.exp(-np.arange(DIM, dtype=np.float64) * np.log(ROPE_BASE) / DIM)
    ang = np.arange(S, dtype=np.float64)[:, None] * freqs[None, :]     # [S, 32]
    cos = np.cos(ang).T                                                # [32, S]
    sin = np.sin(ang).T
    cos64 = np.concatenate([cos, cos], axis=0)                         # [64, S]
    sin64 = np.concatenate([-sin, sin], axis=0)
    cosT = np.concatenate([cos64, cos64], axis=0).astype(BF16)         # [128, S]
    sinT = np.concatenate([sin64, sin64], axis=0).astype(BF16)

    def proj_mat(w, permute, s):
        # rows for one quad stacked [256, 1024] -> [1024, 256] d-major
        blocks = []
        for h in range(4):
            rows = slice(h * HD, (h + 1) * HD)
            wb = w[rows, :]
            if permute:
                wb = wb[perm, :]
            blocks.append(wb * s)
        return np.concatenate(blocks, axis=0).T  # [1024, 256]

    in_maps = []
    for c in range(N_CORES):
        b_ = c // QUADS
        g = c % QUADS
        hs = slice(g * E, (g + 1) * E)
        im = {
            "xqT": np.ascontiguousarray(x_q[b_].T).astype(BF16),     # [1024, S]
            "xkvT": np.ascontiguousarray(x_kv[b_].T).astype(BF16),
            "wqT": np.ascontiguousarray(
                proj_mat(wq[hs, :], True, scale)).astype(BF16),
            "wkT": np.ascontiguousarray(
                proj_mat(wk[hs, :], True, 1.0)).astype(BF16),
            "wvT": np.ascontiguousarray(
                proj_mat(wv[hs, :], False, 1.0)).astype(BF16),
            "woT": np.ascontiguousarray(wo[:, hs].T).astype(BF16),   # [256, 1024]
            "cosT": np.ascontiguousarray(cosT),
            "sinT": np.ascontiguousarray(sinT),
            "ones_col": np.ones((128, 1), dtype=BF16),
        }
        in_maps.append(im)
    return in_maps


# ---------------------------------------------------------------------------
_PROGRAM_CACHE = {}


def _fixed_tile_context(tile_mod, bass_rust_mod, vector_clock_mod):
    """TileContext whose tail drain splits multi-sem waits into single-wait
    NOPs (this walrus rejects >1 sync-wait on one instruction)."""
    SyncInfo = bass_rust_mod.SyncInfo
    ScopedClock = vector_clock_mod.ScopedClock

    class TC(tile_mod.TileContext):
        def _drain_and_barrier(self, tick_clock, wait_clock):
            harvest = self.nc.sync.nop(nofuse=True)
            wait_clock.add_sem_waits(
                harvest.ins, ScopedClock({None: tick_clock.global_clock}))
            si = harvest.ins.sync_info
            waits = list(si.on_wait) if si is not None else []
            if len(waits) > 1:
                harvest.ins.sync_info = SyncInfo(
                    on_wait=[waits[0]], on_update=list(si.on_update))
                for w in waits[1:]:
                    nop = self.nc.sync.nop(nofuse=True)
                    nop.ins.sync_info = SyncInfo(on_wait=[w], on_update=[])
            self.nc.sync.drain()
            self.nc.all_engine_barrier()
            assert self.sems is not None
            popped = self.nc._tile_sem_poison_stack.pop()
            assert popped is self._sem_poison
            self.nc.clear_and_free_semaphores(list(self.sems.allocated().values()))
            self.nc.all_engine_barrier()

    return TC


def _split_multiwait_instructions(nc, mybir, SyncInfo):
    """This walrus build rejects >1 sync-wait per instruction; hoist extra
    waits onto single-wait NOPs inserted just before, on the same engine."""
    ctr = 0
    for blk in nc.m.functions[0].blocks:
        insts = blk.instructions
        i = 0
        while i < len(insts):
            inst = insts[i]
            si = inst.sync_info
            if si is not None and len(si.on_wait) > 1:
                waits = list(si.on_wait)
                inst.sync_info = SyncInfo(on_wait=[waits[-1]],
                                          on_update=list(si.on_update))
                nops = []
                for w in waits[:-1]:
                    nop = mybir.InstNoOp(name=f"waitsplit_{ctr}", ins=[], outs=[])
                    ctr += 1
                    nop.engine = inst.engine
                    nop.sync_info = SyncInfo(on_wait=[w], on_update=[])
                    nops.append(nop)
                insts[i:i] = nops
                i += len(nops)
            i += 1
    return ctr


def build_program(split_waits=True):
    import concourse.bass as bass
    import concourse.mybir as mybir
    import concourse.tile as tile
    import bass_rust
    from concourse import vector_clock

    f32 = mybir.dt.float32
    bf16 = mybir.dt.bfloat16
    i16 = mybir.dt.int16
    Exp = mybir.ActivationFunctionType.Exp
    Copy = mybir.ActivationFunctionType.Copy
    mult = mybir.AluOpType.mult
    add = mybir.AluOpType.add

    nc = bass.Bass("TRN2", target_bir_lowering=False, debug=False,
                   num_devices=N_CORES)

    xqT = nc.dram_tensor("xqT", [D, S], bf16, kind="ExternalInput").ap()
    xkvT = nc.dram_tensor("xkvT", [D, S], bf16, kind="ExternalInput").ap()
    wqT = nc.dram_tensor("wqT", [D, E], bf16, kind="ExternalInput").ap()
    wkT = nc.dram_tensor("wkT", [D, E], bf16, kind="ExternalInput").ap()
    wvT = nc.dram_tensor("wvT", [D, E], bf16, kind="ExternalInput").ap()
    woT = nc.dram_tensor("woT", [E, D], bf16, kind="ExternalInput").ap()
    cosT = nc.dram_tensor("cosT", [128, S], bf16, kind="ExternalInput").ap()
    sinT = nc.dram_tensor("sinT", [128, S], bf16, kind="ExternalInput").ap()
    ones_col = nc.dram_tensor("ones_col", [128, 1], bf16, kind="ExternalInput").ap()
    out = nc.dram_tensor("out", [S, D], f32, kind="ExternalOutput").ap()

    TC = _fixed_tile_context(tile, bass_rust, vector_clock)

    with TC(nc) as tc:
        with tc.tile_pool(name="persist", bufs=1) as per:
            # ---- input DMAs: two HWDGE queues, ordered for eager compute ----
            # sync (SP) queue: wk then xkv per k-tile  -> K projection chases
            wk_sb = per.tile([128, KTILES * E], bf16, tag="wk")
            for k in range(KTILES):
                nc.sync.dma_start(wk_sb[:, k * E:(k + 1) * E],
                                  wkT[k * 128:(k + 1) * 128, :])
            xkv_k = [per.tile([128, S], bf16, tag=f"xkv{k}", name=f"xkv{k}")
                     for k in range(KTILES)]
            for k in range(KTILES):
                nc.sync.dma_start(xkv_k[k][:, :], xkvT[k * 128:(k + 1) * 128, :])

            # scalar (ACT) queue: wv, wq, cos, sin, xq tiles, wo, ones
            wv_sb = per.tile([128, KTILES * E], bf16, tag="wv")
            for k in range(KTILES):
                nc.scalar.dma_start(wv_sb[:, k * E:(k + 1) * E],
                                    wvT[k * 128:(k + 1) * 128, :])
            wq_sb = per.tile([128, KTILES * E], bf16, tag="wq")
            for k in range(KTILES):
                nc.scalar.dma_start(wq_sb[:, k * E:(k + 1) * E],
                                    wqT[k * 128:(k + 1) * 128, :])
            cos_sb = per.tile([128, S], bf16, tag="cos")
            sin_sb = per.tile([128, S], bf16, tag="sin")
            nc.scalar.dma_start(cos_sb[:, :], cosT[:, :])
            nc.scalar.dma_start(sin_sb[:, :], sinT[:, :])
            xq_k = [per.tile([128, S], bf16, tag=f"xq{k}", name=f"xq{k}")
                    for k in range(KTILES)]
            for k in range(KTILES):
                nc.scalar.dma_start(xq_k[k][:, :], xqT[k * 128:(k + 1) * 128, :])
            wo_sb = per.tile([128, 2 * D], bf16, tag="wo")   # pair p at cols p*D
            for p in range(2):
                nc.scalar.dma_start(wo_sb[:, p * D:(p + 1) * D],
                                    woT[p * 128:(p + 1) * 128, :])
            ones_sb = per.tile([128, 1], bf16, tag="ones")
            nc.scalar.dma_start(ones_sb[:, :], ones_col[:, :])

            # persistent activations
            qr_sb = [per.tile([128, S], bf16, tag=f"qr{p}", name=f"qr{p}")
                     for p in range(2)]
            kr_sb = [per.tile([128, S], bf16, tag=f"kr{p}", name=f"kr{p}")
                     for p in range(2)]
            v_sb = per.tile([128, ST * E], bf16, tag="v")    # s-tile st at cols st*E
            ctxu_sb = [per.tile([128, S], bf16, tag=f"ctxu{p}", name=f"ctxu{p}")
                       for p in range(2)]
            ctxn_sb = [per.tile([128, S], bf16, tag=f"ctxn{p}", name=f"ctxn{p}")
                       for p in range(2)]

            # ---- K / Q projections + rope (k-OUTER accumulation) ----
            def qk_projection(w_sb_, x_tiles, dst):
                # dst[p][e(128), s], rope applied.  Both pairs accumulate in
                # PSUM across the k loop (8 banks) so compute chases the DMAs.
                with tc.tile_pool(name="qk_ps", bufs=1, space="PSUM") as pps, \
                     tc.tile_pool(name="qk_tmp", bufs=2) as tmp:
                    q_ps = [pps.tile([128, S], f32, tag=f"qp{p}", name=f"qp{p}")
                            for p in range(2)]
                    for k in range(KTILES):
                        for p in range(2):
                            for sc in range(QC):
                                ss = slice(sc * 512, (sc + 1) * 512)
                                nc.tensor.matmul(
                                    q_ps[p][:, ss],
                                    lhsT=w_sb_[:, k * E + p * 128:
                                               k * E + (p + 1) * 128],
                                    rhs=x_tiles[k][:, ss],
                                    start=(k == 0), stop=(k == KTILES - 1))
                    for p in range(2):
                        qb = tmp.tile([128, S], bf16, tag="qb")
                        qsw = tmp.tile([128, S], bf16, tag="qsw")
                        qcos = tmp.tile([128, S], bf16, tag="qcos")
                        nc.scalar.activation(qb[:, :], q_ps[p][:, :], Copy)
                        # swap halves within each 64-block (partition-swap DMA)
                        for a, bdst in ((0, 32), (32, 0), (64, 96), (96, 64)):
                            nc.sync.dma_start(qsw[bdst:bdst + 32, :],
                                              qb[a:a + 32, :])
                        nc.vector.tensor_tensor(qcos[:, :], qb[:, :],
                                                cos_sb[:, :], mult)
                        nc.vector.tensor_tensor(qsw[:, :], qsw[:, :],
                                                sin_sb[:, :], mult)
                        nc.vector.tensor_tensor(dst[p][:, :], qcos[:, :],
                                                qsw[:, :], add)

            qk_projection(wk_sb, xkv_k, kr_sb)
            qk_projection(wq_sb, xq_k, qr_sb)

            # ---- SDPA (+ V projection and output projection interleaved) ----
            with tc.tile_pool(name="sc_ps", bufs=2, space="PSUM") as scp, \
                 tc.tile_pool(name="cd_ps", bufs=1, space="PSUM") as cdp, \
                 tc.tile_pool(name="v_ps", bufs=1, space="PSUM") as vps, \
                 tc.tile_pool(name="e_sb", bufs=3) as esp, \
                 tc.tile_pool(name="norm", bufs=2) as nrm, \
                 tc.tile_pool(name="o_sb", bufs=3) as osb, \
                 tc.tile_pool(name="ldram", bufs=2, space="DRAM") as ldr:

                def v_proj_tile(st):
                    v_ps = vps.tile([128, E], f32, tag="v_ps")
                    for k in range(KTILES):
                        nc.tensor.matmul(
                            v_ps[:, :],
                            lhsT=xkv_k[k][:, st * 128:(st + 1) * 128],
                            rhs=wv_sb[:, k * E:(k + 1) * E],
                            start=(k == 0), stop=(k == KTILES - 1))
                    nc.vector.tensor_copy(v_sb[:, st * E:(st + 1) * E], v_ps[:, :])

                def o_proj_group(st, ch):
                    # o_ps[s(128), 512] = sum_p ctxn[p][:, st].T @ wo[p, ch]
                    o_ps = scp.tile([128, 1024], f32, tag="s")  # share scores slots
                    cs = slice(ch * 512, (ch + 1) * 512)
                    for p in range(2):
                        nc.tensor.matmul(
                            o_ps[:, 0:512],
                            lhsT=ctxn_sb[p][:, st * 128:(st + 1) * 128],
                            rhs=wo_sb[:, p * D + ch * 512: p * D + (ch + 1) * 512],
                            start=(p == 0), stop=(p == 1))
                    o_out = osb.tile([128, 512], f32, tag="oo")
                    if (st + ch) % 2 == 0:
                        nc.vector.tensor_copy(o_out[:, :], o_ps[:, 0:512])
                    else:
                        nc.scalar.activation(o_out[:, :], o_ps[:, 0:512], Copy)
                    nc.sync.dma_start(out[st * 128:(st + 1) * 128, cs], o_out[:, :])

                def normalize(qh, den_ps):
                    # 1/den on a compacted [128,16] layout so the iterative
                    # reciprocal (8 cyc/elem) costs 0.2us instead of 3.4us;
                    # the DRAM roundtrip doubles as the partition broadcast.
                    qs = slice(qh * 512, (qh + 1) * 512)
                    den_sb = nrm.tile([128, 512], f32, tag="densb")
                    nc.scalar.activation(den_sb[:, :], den_ps[:, :], Copy)
                    dscr = ldr.tile([4, 512], f32, tag="dscr")
                    nc.sync.dma_start(dscr[:, :], den_sb[0:128:32, :])
                    dcmp = nrm.tile([128, 16], f32, tag="dcmp")
                    nc.sync.dma_start(dcmp[:, :], dscr[:, :])
                    linv = nrm.tile([128, 16], f32, tag="linv")
                    nc.vector.reciprocal(linv[:, :], dcmp[:, :])
                    dlin = ldr.tile([4, 512], f32, tag="dlin")
                    nc.sync.dma_start(dlin[:, :], linv[:, :])
                    lbc = [nrm.tile([128, 512], f32, tag=f"lbc{p}",
                                    name=f"lbc{p}") for p in range(2)]
                    for gidx, (p, half) in enumerate(
                            ((0, 0), (0, 1), (1, 0), (1, 1))):
                        nc.sync.dma_start(
                            lbc[p][half * 64:(half + 1) * 64, :],
                            dlin[gidx:gidx + 1, :].partition_broadcast(64))
                    for p in range(2):
                        nc.vector.tensor_tensor(
                            ctxn_sb[p][:, qs], ctxu_sb[p][:, qs],
                            lbc[p][:, :], mult)

                # interleave schedule:
                #  qh 0: V-proj tile per ki
                #  qh 1..3: o-proj groups of qh-1 per ki (8 groups over 16 ki)
                #  after qh 3: remaining o-proj groups of qh 3
                den_prev = None
                for qh in range(QC):
                    qs = slice(qh * 512, (qh + 1) * 512)
                    ctx_ps = [cdp.tile([128, 512], f32, tag=f"ctx{p}",
                                       name=f"ctx{p}") for p in range(2)]
                    den_ps = cdp.tile([128, 512], f32, tag="den")
                    nc.vector.memset(den_ps[:, :], 1.0)
                    for ki in range(KI):
                        ks = slice(ki * 128, (ki + 1) * 128)
                        if qh == 0:
                            v_proj_tile(ki)
                        e_tiles = [None, None]
                        for p in range(2):
                            s_ps = scp.tile([128, 1024], f32, tag="s")
                            nc.tensor.matmul(
                                s_ps[:, 0:512],
                                lhsT=kr_sb[p][0:64, ks], rhs=qr_sb[p][0:64, qs],
                                tile_position=(0, 0), start=True, stop=True)
                            nc.tensor.matmul(
                                s_ps[:, 512:1024],
                                lhsT=kr_sb[p][64:128, ks],
                                rhs=qr_sb[p][64:128, qs],
                                tile_position=(64, 0), start=True, stop=True)
                            e_sb = esp.tile([128, 1024], bf16, tag="e")
                            if p == 0:
                                nc.scalar.activation(e_sb[:, :], s_ps[:, :], Exp)
                            else:
                                nc.vector.tensor_scalar(
                                    e_sb[:, :].bitcast(i16), s_ps[:, :],
                                    float(EXP_A), float(EXP_B - EXP_SIGMA),
                                    mult, add)
                            e_tiles[p] = e_sb
                            nc.tensor.matmul(
                                ctx_ps[p][0:64, :],
                                lhsT=v_sb[:, ki * E + (2 * p) * 64:
                                          ki * E + (2 * p) * 64 + 64],
                                rhs=e_sb[:, 0:512],
                                tile_position=(0, 0),
                                start=(ki == 0), stop=(ki == KI - 1),
                                skip_group_check=True)
                            nc.tensor.matmul(
                                ctx_ps[p][64:128, :],
                                lhsT=v_sb[:, ki * E + (2 * p + 1) * 64:
                                          ki * E + (2 * p + 1) * 64 + 64],
                                rhs=e_sb[:, 512:1024],
                                tile_position=(0, 64),
                                start=(ki == 0), stop=(ki == KI - 1),
                                skip_group_check=True)
                        for gidx, (p, half) in enumerate(
                                ((0, 0), (0, 1), (1, 0), (1, 1))):
                            nc.tensor.matmul(
                                den_ps[gidx * 32: gidx * 32 + 1, :],
                                lhsT=ones_sb[:, :],
                                rhs=e_tiles[p][:, half * 512:(half + 1) * 512],
                                tile_position=(0, gidx * 32),
                                start=(ki == 0), stop=(ki == KI - 1),
                                skip_group_check=True)
                        # interleaved o-proj of the previous chunk
                        if qh >= 1 and 4 <= ki < 12:
                            g = ki - 4
                            o_proj_group((qh - 1) * 4 + g // 2, g % 2)
                    # unnormalized ctx out of PSUM (frees banks fast)
                    for p in range(2):
                        nc.scalar.activation(ctxu_sb[p][:, qs],
                                             ctx_ps[p][:, :], Copy)
                    normalize(qh, den_ps)
                # tail: o-proj for the last chunk
                for st in range(12, 16):
                    for ch in range(2):
                        o_proj_group(st, ch)

    if split_waits:
        _split_multiwait_instructions(nc, mybir, bass_rust.SyncInfo)
    return nc


def kernel(x_q, x_kv, wq, bq, wk, bk, wv, bv, wo, bo):
    from concourse import bass_utils

    x_q = np.asarray(x_q, dtype=np.float32)
    x_kv = np.asarray(x_kv, dtype=np.float32)
    wq = np.asarray(wq, dtype=np.float32); bq = np.asarray(bq, dtype=np.float32)
    wk = np.asarray(wk, dtype=np.float32); bk = np.asarray(bk, dtype=np.float32)
    wv = np.asarray(wv, dtype=np.float32); bv = np.asarray(bv, dtype=np.float32)
    wo = np.asarray(wo, dtype=np.float32); bo = np.asarray(bo, dtype=np.float32)

    out_bias = bo.copy()
    if np.any(bv):
        # ctx = attn @ (v + bv) = attn @ v + bv (attn rows sum to 1), and bv
        # maps through wo onto the host-side output bias.
        out_bias += wo @ bv
    if np.any(bq) or np.any(bk):
        # q/k biases rotate with RoPE, so fold them into the inputs instead:
        # (x + c) @ w.T == x @ w.T + b  when  w @ c == b  (w square,
        # generically invertible).  The k-fold also perturbs v by wv @ ck,
        # which maps through wo and is subtracted on the host.
        cq = np.linalg.solve(wq.astype(np.float64),
                             bq.astype(np.float64)).astype(np.float32)
        ck = np.linalg.solve(wk.astype(np.float64),
                             bk.astype(np.float64)).astype(np.float32)
        x_q = x_q + cq[None, None, :]
        x_kv = x_kv + ck[None, None, :]
        out_bias -= wo @ (wv @ ck)

    in_maps = _host_prep(x_q, x_kv, wq, bq, wk, bk, wv, bv, wo)

    if "prog" not in _PROGRAM_CACHE:
        _PROGRAM_CACHE["prog"] = build_program()
    nc = _PROGRAM_CACHE["prog"]

    res = bass_utils.run_bass_kernel_spmd(
        nc, in_maps, core_ids=list(range(N_CORES)),
        trace=os.environ.get("KERNEL_TRACE", "") == "1")
    _PROGRAM_CACHE["last_result"] = res

    out = np.zeros((B, S, D), dtype=np.float32)
    for c in range(N_CORES):
        out[c // QUADS] += res.results[c]["out"]
    out += out_bias[None, None, :]
    return out
